# revision 56
# baseline (speedup 1.0000x reference)
"""Block-sparse attention on 8 trn2 cores: fp8 + DoubleRow quadrant matmuls.

Reference semantics (B=1, H=16, S=4096, D=64, BLOCK=64):
    scores  = (Q @ K^T) / 8;  scores *= blockdiag mask (off-block -> 0)
    weights = softmax(scores, axis=-1) over the FULL row;  out = weights @ V

Off-block entries contribute exp(0)=1 to the softmax, so with
num_q = sum_{k in blk} e_qk v_k and den_q = sum_{k in blk} e_qk:
    out_q = (num_q + Vtot - Vblk) / (den_q + S - 64)
The device computes ONLY the block-local num|den; the host (assemble)
applies the O(S*D) corrections, the 64x fp8 scale, and the divide in fp32.

Sharding: head-parallel, 2 heads/core, packed as PE quadrants.

Design (fp8 inputs, ~2.6x less HBM traffic than the bf16 baseline):
  - qk host layout [128, 2, 2, S/2] fp8: hybrid DoubleRow. DR outputs must
    land at PSUM partition 0 (col_grp 0xf), so only head0 can use DR: its
    d=64 contraction splits as 32 partitions x 2 free planes (partitions
    0:32 = seq-half 0, 32:64 = half 1, e4m3 bits) -> 32-cycle DR matmuls
    into psum rows 0:64. head1 ships flat (partitions 64:128 = d, free
    (t, hf, s'), e3m4 bits read via bitcast) -> plain 64-cycle matmuls
    into psum rows 64:128. Blocks are processed in interleaved pos order
    (pos=2j+hf -> block j+32*hf) so every slab spans both seq halves and
    per-partition DMA bytes stay minimal; host unscrambles rows.
  - exp on ACT per group [128, n<=16, 64] psum->SBUF bf16 (et = E^T).
  - num = et.T @ V: one matmul per head-block (lhsT=et bf16, rhs=V fp8e3),
    out [64 q, 64 d] psum [128, n, 64] (2 banks).
  - den via rhs=ones into col 0 of the scores psum tile right after exp
    consumed it; den evac (ACT) frees that tile for scores(g+2).
  - evacs scale by 1/64 into fp8e3 (num/den reach ~620/~330, e3m4 max 15.5;
    host multiplies back) -- e3m4's 4 mantissa bits keep l2 err ~1.7e-2.
  - out DRAM [128, 68, 64] fp8e3: rows 0:64 = num, row 64 = den row,
    rows 65:68 pad so the partition stride (68*64B) is 256B-divisible for
    the final dma_scatter_add.
  - rows 36:65 (late blocks + den row) ship via dma_scatter_add ops riding
    the Pool engine (dest pre-zeroed by an early DMA), avoiding the ~2.2us
    dma_start latency+descriptor tail after their evacs; rows 0:36 go as
    bulk dma_start stores (Pool then SP) whose latency hides mid-stream.
  - DMA: qk in 6 slabs on SP, v in 3 slabs on Pool (SWDGE); per-engine
    DMA streams overlap.
  - PE p-state is absolute-time in the cost model (MID till 3us, FULL
    after); no warmup matmul needed.
"""

import numpy as np

H, S, D = 16, 4096, 64
HPC = 2
NCORES = 8
NBLK = 64
SCALE = 0.125

GROUPS = (4, 8, 16, 16, 13, 4, 3)
QK_SLABS = ((0, 4), (4, 12), (12, 28), (28, 44), (44, 58), (58, 64))
V_SLABS = ((0, 24), (24, 48), (48, 64))
O_SLABS = ((0, 22), (22, 36))  # bulk DMA stores
SCATTERS = ((36, 44), (44, 57), (57, 65))  # rows via scatter-add (incl den row)
# evac engine per 8-block chunk, in block order ("v"=DVE, "a"=ACT)
EVAC_ENG = ("v", "v", "v", "v", "v", "v", "v", "a", "v", "v", "v")
DEN_ENG = "a"
O_ENG = ("p", "s")  # per bulk store: "s"=SP, "p"=Pool
OPAD = 68
ESCALE = 1.0 / 64

_CACHE = {}
OUT_NAMES = ["out"]


def _build_bass():
    import concourse.bass as bass
    import concourse.bacc as bacc
    import concourse.tile as tile
    from concourse import mybir

    f32 = mybir.dt.float32
    bf16 = mybir.dt.bfloat16
    i16 = mybir.dt.int16
    f8e4 = mybir.dt.float8e4
    f8e3 = mybir.dt.float8e3
    EXP = mybir.ActivationFunctionType.Exp
    DR = mybir.MatmulPerfMode.DoubleRow

    nc = bacc.Bacc(
        "TRN2", target_bir_lowering=False, debug=False, num_devices=NCORES
    )
    qk_d = nc.dram_tensor("qk", [128, 2, 2, S // 2], f8e4, kind="ExternalInput")
    v_d = nc.dram_tensor("value", [128, NBLK, D], f8e3, kind="ExternalInput")
    si_d = nc.dram_tensor("sidx", [128, 8], i16, kind="ExternalInput")
    o_d = nc.dram_tensor("out", [128, OPAD, D], f8e3, kind="ExternalOutput")

    goff = [0]
    for n in GROUPS:
        goff.append(goff[-1] + n)
    assert goff[-1] == NBLK
    NGRP = len(GROUPS)

    with tile.TileContext(nc) as tc:
        with (
            tc.tile_pool(name="consts", bufs=1) as consts,
            tc.tile_pool(name="io", bufs=1) as io,
            tc.tile_pool(name="work", bufs=3) as work,
            tc.tile_pool(name="ps_s", bufs=2, space="PSUM") as ps_s,
            tc.tile_pool(name="ps_n", bufs=3, space="PSUM") as ps_n,
            tc.tile_pool(name="ps_d", bufs=1, space="PSUM") as ps_d,
        ):
            # preload the exp table on ACT at t~0 (InstLoadActFuncSet is 1283ns)
            dummy = consts.tile([1, 1], f32, tag="dummy")
            nc.gpsimd.memset(dummy, 0.0)
            nc.scalar.activation(out=dummy, in_=dummy, func=EXP, scale=1.0)
            ones = consts.tile([128, 1], bf16, tag="ones")
            nc.vector.memset(ones, 1.0)
            # zeros for the scatter-add destination rows
            zlo = min(r0 for r0, _ in SCATTERS)
            zhi = max(r1 for _, r1 in SCATTERS)
            zt = consts.tile([128, zhi - zlo, D], f8e3, tag="zt")
            it16 = consts.tile([128, 8], i16, tag="it")

            qks = [
                io.tile(
                    [128, 2, 2, (b - a) // 2 * D], f8e4, tag=f"qk{s}", name=f"qk{s}"
                )
                for s, (a, b) in enumerate(QK_SLABS)
            ]
            vhs = [
                io.tile([128, b - a, D], f8e3, tag=f"vh{s}", name=f"vh{s}")
                for s, (a, b) in enumerate(V_SLABS)
            ]
            oh = io.tile([128, NBLK + 1, D], f8e3, tag="oh")

            for s, (a, b) in enumerate(QK_SLABS):
                nc.sync.dma_start(
                    out=qks[s], in_=qk_d[:, :, :, a // 2 * D : b // 2 * D]
                )
            for s, (a, b) in enumerate(V_SLABS):
                nc.gpsimd.dma_start(out=vhs[s], in_=v_d[:, a:b, :])
            nc.gpsimd.memset(zt, 0.0)
            # late consts for the scatter-add tail (no deps upstream)
            nc.sync.dma_start(out=it16, in_=si_d[:, :])
            nc.sync.dma_start(
                out=o_d[:, zlo:zhi, :], in_=zt[:, 0 : zhi - zlo, :]
            )

            def qkblk(pos):
                for s, (a, b) in enumerate(QK_SLABS):
                    if a <= pos < b:
                        lo = (pos - a) // 2 * D
                        return qks[s], slice(lo, lo + D), pos % 2
                raise AssertionError

            def vblk(blk):
                for s, (a, b) in enumerate(V_SLABS):
                    if a <= blk < b:
                        return vhs[s], blk - a
                raise AssertionError

            den = ps_d.tile([128, NBLK, 1], f32, tag="den")
            sco = {}

            def scores(g):
                n = GROUPS[g]
                pss = ps_s.tile([128, n, D], f32, tag="ps", name=f"ps_{g}")
                sco[g] = pss
                for i in range(n):
                    qk, cols, hf = qkblk(goff[g] + i)
                    base = 32 * hf
                    nc.tensor.matmul(  # head0: DoubleRow, dst psum rows 0:64
                        pss[0:64, i, :],
                        qk[base : base + 32, 1, :, cols],
                        qk[base : base + 32, 0, :, cols],
                        start=True,
                        stop=True,
                        perf_mode=DR,
                        tile_position=(base, 0),
                    )
                    nc.tensor.matmul(  # head1: plain, dst psum rows 64:128
                        pss[64:128, i, :],
                        qk[64:128, 1, hf, cols].bitcast(f8e3),
                        qk[64:128, 0, hf, cols].bitcast(f8e3),
                        start=True,
                        stop=True,
                        tile_position=(64, 64),
                    )

            nst = [0]

            def store_ready(blocks_done):
                while nst[0] < len(O_SLABS):
                    s0, s1 = O_SLABS[nst[0]]
                    if s1 > blocks_done:
                        break
                    eng = nc.sync if O_ENG[nst[0]] == "s" else nc.gpsimd
                    eng.dma_start(out=o_d[:, s0:s1, :], in_=oh[:, s0:s1, :])
                    nst[0] += 1

            nchunk = [0]
            scores(0)
            for g in range(NGRP):
                if g + 1 < NGRP and (g + 1) not in sco:
                    scores(g + 1)
                # pull the small tail groups' scores ahead of the PE queue
                if g == NGRP - 3 and (g + 2) not in sco:
                    scores(g + 2)
                n = GROUPS[g]
                b0 = goff[g]
                pss = sco.pop(g)
                et = work.tile([128, n, D], bf16, tag="et", name=f"et_{g}")
                # exp + nums in <=8-block chunks (ps_n tiles are 1 psum bank):
                # chunked exp lets each chunk's nums start ~0.5us earlier.
                # den accumulates in its own persistent bank, evac'd once
                for c0 in range(0, n, 8):
                    cn = min(8, n - c0)
                    nc.scalar.activation(
                        out=et[:, c0 : c0 + cn, :],
                        in_=pss[:, c0 : c0 + cn, :],
                        func=EXP,
                        scale=SCALE,
                    )
                    num = ps_n.tile(
                        [128, cn, D], f32, tag="pn", name=f"pn_{g}_{c0}"
                    )
                    for i in range(cn):
                        b = b0 + c0 + i
                        vt, vi = vblk(b)
                        for lo, hi in ((0, 64), (64, 128)):
                            nc.tensor.matmul(
                                num[lo:hi, i, :],
                                et[lo:hi, c0 + i, :],
                                vt[lo:hi, vi, :],
                                start=True,
                                stop=True,
                            )
                            nc.tensor.matmul(
                                den[lo:hi, b, :],
                                et[lo:hi, c0 + i, :],
                                ones[lo:hi, :],
                                start=True,
                                stop=True,
                            )
                    osl = oh[:, b0 + c0 : b0 + c0 + cn, :]
                    if EVAC_ENG[nchunk[0]] == "v":
                        nc.vector.tensor_scalar_mul(out=osl, in0=num, scalar1=ESCALE)
                    else:
                        nc.scalar.mul(out=osl, in_=num, mul=ESCALE)
                    nchunk[0] += 1
                store_ready(b0 + n)

            # den row evac (once), then the late rows ship via scatter-adds
            # riding the Pool engine -- no dma_start latency tail
            dsl = oh[:, NBLK : NBLK + 1, :].rearrange("p one n -> p (one n)")
            din = den.rearrange("p b one -> p (b one)")
            if DEN_ENG == "a":
                nc.scalar.mul(out=dsl, in_=din, mul=ESCALE)
            else:
                nc.vector.tensor_scalar_mul(out=dsl, in0=din, scalar1=ESCALE)
            for r0, r1 in SCATTERS:
                nc.gpsimd.dma_scatter_add(
                    out_ap=o_d[:, r0:r1, :].rearrange("p a b -> p (a b)").rearrange(
                        "p (t e) -> p t e", t=1
                    ),
                    in_ap=oh[:, r0:r1, :].rearrange("p a b -> p (a b)").rearrange(
                        "p (t e) -> p t e", t=1
                    ),
                    idxs_ap=it16[:, :],
                    num_idxs=128,
                    num_idxs_reg=128,
                    elem_size=(r1 - r0) * D,
                    elem_step=OPAD * D,
                )

    nc.compile()
    return nc


def _get_compiled():
    if "nc" not in _CACHE:
        _CACHE["nc"] = _build_bass()
    return _CACHE["nc"]


def make_in_maps(query, key, value):
    import ml_dtypes

    E4 = ml_dtypes.float8_e4m3
    E3 = ml_dtypes.float8_e3m4
    q = np.asarray(query, dtype=np.float32).reshape(H, S, D)
    k = np.asarray(key, dtype=np.float32).reshape(H, S, D)
    v = np.asarray(value, dtype=np.float32).reshape(H, S, D)
    in_maps = []
    pos_block = [p // 2 + 32 * (p % 2) for p in range(NBLK)]
    for c in range(NCORES):
        qk = np.empty((128, 2, 2, S // 2), dtype=E4)
        h0, h1 = HPC * c, HPC * c + 1
        for hf in range(2):
            ssl = slice(2048 * hf, 2048 * hf + 2048)
            for g in range(2):
                dsl = slice(32 * g, 32 * g + 32)
                qk[32 * hf : 32 * hf + 32, 0, g, :] = q[h0].T[dsl, ssl].astype(E4)
                qk[32 * hf : 32 * hf + 32, 1, g, :] = k[h0].T[dsl, ssl].astype(E4)
            # head1 flat rows hold e3m4 BITS (device bitcasts the slice)
            qk[64:128, 0, hf, :] = q[h1].T[:, ssl].astype(E3).view(E4)
            qk[64:128, 1, hf, :] = k[h1].T[:, ssl].astype(E3).view(E4)
        hs = slice(HPC * c, HPC * c + HPC)
        vb = v[hs].reshape(2, NBLK, D, D).transpose(0, 2, 1, 3).reshape(128, NBLK, D)
        vb = vb[:, pos_block, :]
        sidx = (
            (np.arange(128) % 16)[:, None] + 16 * np.arange(8)[None, :]
        ).astype(np.int16)
        in_maps.append(
            {
                "qk": np.ascontiguousarray(qk),
                "value": np.ascontiguousarray(vb.astype(E3)),
                "sidx": sidx,
            }
        )
    return in_maps


def run_spmd(in_maps, **kwargs):
    from concourse.bass_utils import run_bass_kernel_spmd

    nc = _get_compiled()
    return run_bass_kernel_spmd(nc, in_maps, core_ids=list(range(NCORES)), **kwargs)


def assemble(res, value):
    """Host correction: out = (num + Vtot - Vb) / (den + S - 64)."""
    v = np.asarray(value, dtype=np.float32).reshape(H, S, D)
    vb = v.reshape(H, NBLK, D, D).sum(axis=2)  # [H, 64, D]
    vtot = vb.sum(axis=1)  # [H, D]
    w = (vtot[:, None, :] - vb).astype(np.float32)  # [H, 64, D]

    pos_block = [p // 2 + 32 * (p % 2) for p in range(NBLK)]
    inv = np.argsort(pos_block)
    out = np.empty((H, S, D), dtype=np.float32)
    for c in range(NCORES):
        o = np.asarray(res.results[c]["out"], dtype=np.float32)  # [128, 68, 64]
        o = o.reshape(2, D, OPAD, D) * np.float32(1.0 / ESCALE)
        for hh in range(2):
            h = 2 * c + hh
            num = o[hh, :, inv, :].transpose(1, 0, 2) + w[h][None, :, :]
            den = o[hh, :, NBLK, :][:, inv] + np.float32(S - 64)  # [r, b]
            out[h] = (num / den[:, :, None]).transpose(1, 0, 2).reshape(S, D)
    return out.reshape(1, H, S, D)


def kernel(query: np.ndarray, key: np.ndarray, value: np.ndarray) -> np.ndarray:
    return assemble(run_spmd(make_in_maps(query, key, value)), value)
